# revision 23
# baseline (speedup 1.0000x reference)
"""MiniAttentionBlock (LayerNorm -> causal MHA -> out-proj + residual) on 8 trn2 cores.

Sharding: core i handles batch b=i//2, head-group g=i%2 (4 heads = 512 features).
Each core returns a partial [T, H] = attnout(4 heads) @ Wo[:, slice].T  (no residual);
the host sums the two partials per batch and adds the residual x.

On-core pipeline (activations feature-major [feat, token]; x/weights/probs/V in
bf16, scores+out-proj in f32r):
  1. stats: mean/meansq via ones-matmuls on PE -> mu, std (stt2), rstd_b
     (broadcast rows), rstd_col (token-major rstd via inv128 ones-matmul),
     sc_col = SCALE*rstd.
  2. QKV = W@x matmuls (raw x, bf16) + rank-2 aux@[mu; std] correction;
     LayerNorm rstd is folded into the PSUM evacuation: Q *= rstd (free-dim
     broadcast tile), K copied plain (k-side rstd goes into the exp scale),
     V *= rstd (per-partition scalar).  QKV matmuls need no stats.
  3. attention, qc-outer, kt-outer, head-inner (pipelines PE/ACT/DVE):
     scoresT[k,q] (f32r) -> exp on ACT with per-partition scale SCALE*rstd[k]
     -> bf16 probs -> causal affine_select only on the 128x128 diagonal block
     -> A@V (bf16, PSUM accum) -> denominator via alternating bf16 DVE adds
     + ones-matmul partition-reduce into one shared PSUM bank (rows 32h)
     -> one strided reciprocal for 4 heads -> normalize via K=1 broadcast
     matmul + DVE mul from two PSUM banks.
     Upper-triangle work is trimmed: diagonal k-tiles only compute the
     q-subrange >= k.
  4. y = attnout^T-stationary f32r matmul with WoT -> Pool-engine PSUM
     evacuation (bf16) -> DMA out; emitted one qc late for slack.
"""

import numpy as np

H = 1024
T = 2048
B = 4
NCORES = 8
D = 128          # head dim
HPC = 4          # heads per core
F = HPC * D      # 512 out features per core
NC_CHUNKS = H // 128   # 8 feature chunks
NT = T // 128          # 16 token tiles
NQ = T // 512          # 4 token chunks of 512
SCALE = float(D) ** -0.5

_CACHED = {}


def _build_program():
    import concourse.bass as bass
    import concourse.tile as tile
    from concourse import bacc, mybir
    from concourse.bass import ts

    f32 = mybir.dt.float32
    f32r = mybir.dt.float32r
    bf16 = mybir.dt.bfloat16
    f8 = mybir.dt.float8e4
    DR = mybir.MatmulPerfMode.DoubleRow
    AL = mybir.AluOpType
    ACTF = mybir.ActivationFunctionType

    nc = bacc.Bacc("TRN2", target_bir_lowering=False, debug=False, num_devices=NCORES)

    xT = nc.dram_tensor("xT", [H, T], bf16, kind="ExternalInput").ap()
    wqT = nc.dram_tensor("wqT", [H, F], bf16, kind="ExternalInput").ap()
    wkT = nc.dram_tensor("wkT", [H, F], bf16, kind="ExternalInput").ap()
    wvT = nc.dram_tensor("wvT", [H, F], bf16, kind="ExternalInput").ap()
    woT = nc.dram_tensor("woT", [F, H], bf16, kind="ExternalInput").ap()
    auxq = nc.dram_tensor("auxq", [2, F], f32r, kind="ExternalInput").ap()
    auxk = nc.dram_tensor("auxk", [2, F], f32r, kind="ExternalInput").ap()
    auxv = nc.dram_tensor("auxv", [2, F], f32r, kind="ExternalInput").ap()
    out = nc.dram_tensor("out", [T, H], bf16, kind="ExternalOutput").ap()

    with tile.TileContext(nc) as tc:
        with tc.tile_pool(name="persist", bufs=1) as persist:
            zero_col = persist.tile([128, 1], f32)
            nc.vector.memset(zero_col, 0.0)
            eps_sb = persist.tile([1, 1], f32)
            nc.vector.memset(eps_sb, 1e-5)
            ones_col_bf = persist.tile([128, 1], bf16)
            nc.vector.memset(ones_col_bf, 1.0)
            inv128_bf = persist.tile([128, 1], bf16)
            nc.vector.memset(inv128_bf, 1.0 / 128.0)
            # stt2: row0 = mean/H, row1 = std (written by stats)
            stt2 = persist.tile([2, T], f32r)
            aq_sb = persist.tile([2, F], f32r, tag="aq")
            ak_sb = persist.tile([2, F], f32r, tag="ak")
            av_sb = persist.tile([2, F], f32r, tag="av")
            nc.sync.dma_start(out=aq_sb, in_=auxq)
            nc.sync.dma_start(out=ak_sb, in_=auxk)
            nc.sync.dma_start(out=av_sb, in_=auxv)
            qT_all = persist.tile([128, HPC, T], f32r, tag="qT")
            kT_all = persist.tile([128, HPC, T], f32r, tag="kT")
            v_all = persist.tile([128, NT, F], bf16, tag="v")
            rstd_col = persist.tile([128, NT], f32)
            sc_col = persist.tile([128, NT], f32)
            wo_sb = persist.tile([128, HPC, H], bf16, tag="wo")

            with tc.tile_pool(name="xtp", bufs=1) as xtp:
                xt = xtp.tile([128, NC_CHUNKS, T], bf16)
                rstd_b = xtp.tile([128, T], f32r)
                xT_r = xT.rearrange("(c p) t -> p c t", p=128)
                # tq-major loads so stats on the first 512 tokens start early
                for tq in range(NQ):
                    for c in range(NC_CHUNKS):
                        eng = nc.sync if c % 2 == 0 else nc.gpsimd
                        eng.dma_start(
                            out=xt[:, c, ts(tq, 512)],
                            in_=xT_r[:, c, ts(tq, 512)],
                        )
                nc.sync.dma_start(
                    out=wo_sb, in_=woT.rearrange("(c p) n -> p c n", p=128)
                )

                # ---- phase 1+2: stats and QKV (stats hidden under QKV) -------
                with (
                    tc.tile_pool(name="sqp", bufs=12) as sqp,
                    tc.tile_pool(name="stats", bufs=2) as stats,
                    tc.tile_pool(name="ps1", bufs=1, space="PSUM") as ps1,
                    tc.tile_pool(name="ps2", bufs=4, space="PSUM") as ps2,
                ):
                    mean_pss = {}
                    sq_tiles = {}
                    sq_pss = {}

                    def emit_mean(tq):
                        sl = ts(tq, 512)
                        mean_ps = ps1.tile([1, 512], f32, tag="mean", bufs=1)
                        for c in range(NC_CHUNKS):
                            nc.tensor.matmul(
                                mean_ps, ones_col_bf, xt[:, c, sl],
                                start=(c == 0), stop=(c == NC_CHUNKS - 1),
                            )
                        mean_pss[tq] = mean_ps

                    def emit_sq_act(tq):
                        sl = ts(tq, 512)
                        tls = []
                        for c in range(NC_CHUNKS):
                            sq_t = sqp.tile([128, 512], bf16, tag="sqt")
                            nc.scalar.activation(
                                sq_t, xt[:, c, sl], ACTF.Square, bias=zero_col
                            )
                            tls.append(sq_t)
                        sq_tiles[tq] = tls

                    def emit_sqmm(tq):
                        sq_ps = ps1.tile([1, 512], f32, tag="sq", bufs=1)
                        for c in range(NC_CHUNKS):
                            nc.tensor.matmul(
                                sq_ps, ones_col_bf, sq_tiles[tq][c],
                                start=(c == 0), stop=(c == NC_CHUNKS - 1),
                            )
                        sq_pss[tq] = sq_ps

                    def emit_chain(tq):
                        sl = ts(tq, 512)
                        mean_ps = mean_pss[tq]
                        sq_ps = sq_pss[tq]
                        # stt2 row0 = mean/H
                        nc.vector.tensor_scalar_mul(stt2[0:1, sl], mean_ps, 1.0 / H)
                        # spre = (mean/H)^2
                        spre = stats.tile([1, 512], f32, tag="spre")
                        nc.vector.tensor_mul(spre, stt2[0:1, sl], stt2[0:1, sl])
                        # var = meansq/H - spre
                        varr = stats.tile([1, 512], f32, tag="varr")
                        nc.vector.scalar_tensor_tensor(
                            varr, sq_ps, 1.0 / H, spre,
                            op0=AL.mult, op1=AL.subtract,
                        )
                        # stt2 row1 = std = sqrt(var + eps); ACT can't write at
                        # partition offset 1, so bounce through a DMA
                        std_tmp = stats.tile([1, 512], f32r, tag="stdt")
                        nc.scalar.activation(std_tmp, varr, ACTF.Sqrt, bias=eps_sb)
                        nc.gpsimd.dma_start(out=stt2[1:2, sl], in_=std_tmp)
                        rstd = stats.tile([1, 512], f32r, tag="rstd")
                        with nc.allow_low_precision(reason="tf32 rstd"):
                            nc.vector.reciprocal(rstd, std_tmp)
                        # broadcast rstd to 128 partitions on Pool
                        nc.gpsimd.partition_broadcast(rstd_b[:, sl], rstd)
                        # token-major rstd columns: bf16 broadcast, then a
                        # 1-wide ones-matmul per 128-token block (transpose)
                        rstd_bf = stats.tile([1, 512], bf16, tag="rbf")
                        nc.vector.tensor_copy(rstd_bf, rstd)
                        rstd_bb = stats.tile([128, 512], bf16, tag="rbb")
                        nc.gpsimd.partition_broadcast(rstd_bb, rstd_bf)
                        for i in range(4):
                            tp_ps = ps1.tile([128, 1], f32, tag="bc", bufs=1, name="tp_ps")
                            nc.tensor.matmul(
                                tp_ps, rstd_bb[:, ts(i, 128)], inv128_bf,
                                start=True, stop=True,
                            )
                            nc.vector.tensor_copy(
                                rstd_col[:, 4 * tq + i : 4 * tq + i + 1], tp_ps
                            )
                        nc.vector.tensor_scalar_mul(
                            sc_col[:, 4 * tq : 4 * tq + 4],
                            rstd_col[:, 4 * tq : 4 * tq + 4], SCALE,
                        )

                    emit_mean(0)
                    emit_sq_act(0)
                    emit_mean(1)
                    emit_sq_act(1)
                    emit_sqmm(0)
                    emit_chain(0)
                    emit_mean(2)
                    emit_sq_act(2)
                    emit_sqmm(1)
                    emit_chain(1)
                    emit_mean(3)
                    emit_sq_act(3)
                    emit_sqmm(2)
                    emit_chain(2)
                    emit_sqmm(3)
                    emit_chain(3)

                    # ---- QKV (fp8 DoubleRow) --------------------------------
                    with (
                        tc.tile_pool(name="wqk", bufs=3) as wqk,
                        tc.tile_pool(name="wvp", bufs=1) as wvp,
                    ):
                        def emit_qk_group(w_t, aux_sb, dst, mi, tq, is_q):
                            sl = ts(tq, 512)
                            ps = ps2.tile([128, 512], f32, tag="qk", name="ps")
                            for c in range(NC_CHUNKS):
                                nc.tensor.matmul(
                                    ps, w_t[:, c, :], xt[:, c, sl],
                                    start=(c == 0), stop=False,
                                )
                            nc.tensor.matmul(
                                ps, aux_sb[:, ts(mi, 128)], stt2[:, sl],
                                start=False, stop=True,
                            )
                            if is_q:
                                nc.vector.tensor_mul(dst[:, mi, sl], ps, rstd_b[:, sl])
                            else:
                                nc.scalar.copy(dst[:, mi, sl], ps)

                        def load_w(wT, mi):
                            w_t = wqk.tile([128, NC_CHUNKS, 128], bf16, tag="w", name="w_t")
                            nc.sync.dma_start(
                                out=w_t,
                                in_=wT.rearrange("(c p) m -> p c m", p=128)[
                                    :, :, ts(mi, 128)
                                ],
                            )
                            return w_t

                        wq0 = load_w(wqT, 0)
                        for tq in range(NQ):
                            emit_qk_group(wq0, aq_sb, qT_all, 0, tq, True)
                        for wT, aux_sb, dst, mi0, is_q in (
                            (wqT, aq_sb, qT_all, 1, True),
                            (wkT, ak_sb, kT_all, 0, False),
                        ):
                            for mi in range(mi0, HPC):
                                w_t = load_w(wT, mi)
                                for tq in range(NQ):
                                    emit_qk_group(w_t, aux_sb, dst, mi, tq, is_q)
                        # V: token-major, full 512-wide, DoubleRow
                        wv_t = wvp.tile([128, NC_CHUNKS, F], bf16, tag="wv")
                        nc.sync.dma_start(
                            out=wv_t, in_=wvT.rearrange("(c p) m -> p c m", p=128)
                        )
                        for ti in range(NT):
                            tsl = ts(ti, 128)
                            ps = ps2.tile([128, 512], f32, tag="qk")
                            for c in range(NC_CHUNKS):
                                nc.tensor.matmul(
                                    ps, xt[:, c, tsl], wv_t[:, c, :],
                                    start=(c == 0), stop=False,
                                )
                            nc.tensor.matmul(
                                ps, stt2[:, tsl], av_sb, start=False, stop=True
                            )
                            nc.vector.tensor_scalar(
                                v_all[:, ti, :], ps, rstd_col[:, ti : ti + 1],
                                None, op0=AL.mult,
                            )

            # ---- phase 3+4: attention + out projection ----------------------
            with (
                tc.tile_pool(name="atp", bufs=1) as atp,
                tc.tile_pool(name="probs", bufs=10) as probs,
                tc.tile_pool(name="dnp", bufs=4) as dnp,
                tc.tile_pool(name="rdp", bufs=8) as rdp,
                tc.tile_pool(name="rbp", bufs=4) as rbp,
                tc.tile_pool(name="yp", bufs=4) as yp,
                tc.tile_pool(name="psSP", bufs=4, space="PSUM") as psSP,
                tc.tile_pool(name="psAV", bufs=4, space="PSUM") as psAV,
            ):
                at_all = atp.tile([128, HPC, T], bf16)

                def emit_y_group(ti, hc):
                    tsl = ts(ti, 128)
                    hsl = ts(hc, 512)
                    y_ps = psSP.tile([128, 512], f32, tag="sp", name="y_ps")
                    for c in range(HPC):
                        nc.tensor.matmul(
                            y_ps, at_all[:, c, tsl], wo_sb[:, c, hsl],
                            start=(c == 0), stop=(c == HPC - 1),
                        )
                    y_sb = yp.tile([128, 512], bf16, tag="ysb", name="y_sb")
                    nc.vector.tensor_copy(y_sb, y_ps)
                    nc.sync.dma_start(out=out[tsl, hsl], in_=y_sb)

                for qc in range(NQ):
                    qsl = ts(qc, 512)
                    nk = 4 * qc + 4
                    avs = [psAV.tile([128, 512], f32, tag="av", name="av") for _ in range(HPC)]
                    dn0s = [dnp.tile([128, 512], bf16, tag="dn0", name="dn0") for _ in range(HPC)]
                    dn1s = (
                        [dnp.tile([128, 512], bf16, tag="dn1", name="dn1") for _ in range(HPC)]
                        if qc > 0
                        else None
                    )
                    ygroups = (
                        [(ti, hc) for ti in range(4 * (qc - 1), 4 * qc) for hc in range(2)]
                        if qc > 0
                        else []
                    )

                    def emit_dn(h, kt, off, pt, psl):
                        if kt == 0:
                            nc.vector.tensor_copy(dn0s[h], pt[:, psl])
                        elif qc == 0:
                            nc.vector.tensor_add(
                                dn0s[h][:, off:], dn0s[h][:, off:], pt[:, psl]
                            )
                        elif kt == 1:
                            nc.vector.tensor_copy(dn1s[h], pt[:, psl])
                        else:
                            dnx = dn0s[h] if kt % 2 == 0 else dn1s[h]
                            nc.vector.tensor_add(
                                dnx[:, off:], dnx[:, off:], pt[:, psl]
                            )

                    def emit_avdn(round_items):
                        kt, off, pts = round_items
                        for h in range(HPC):
                            nc.tensor.matmul(
                                avs[h][:, off:],
                                v_all[:, kt, ts(h, 128)],
                                pts[h][:, off:],
                                start=(kt == 0), stop=(kt == nk - 1),
                                skip_group_check=True,
                            )
                        for h in range(HPC):
                            emit_dn(h, kt, off, pts[h], slice(off, 512))

                    prev = None
                    for kt in range(nk):
                        diag = kt - 4 * qc
                        off = 128 * diag if diag >= 0 else 0
                        s_off = min(off, 256)
                        pts = []
                        for h in range(HPC):
                            qh = qT_all[:, h, :]
                            kh = kT_all[:, h, :]
                            sp = psSP.tile([128, 512], f32, tag="sp", name="sp")
                            nc.tensor.matmul(
                                sp[:, s_off:],
                                kh[:, ts(kt, 128)],
                                qh[:, 512 * qc + s_off : 512 * (qc + 1)],
                                start=True, stop=True, skip_group_check=True,
                            )
                            pt = probs.tile([128, 512], bf16, tag="pt")
                            nc.scalar.activation(
                                pt[:, off:], sp[:, off:], ACTF.Exp,
                                bias=zero_col, scale=sc_col[:, kt : kt + 1],
                            )
                            if diag >= 0:
                                nc.gpsimd.affine_select(
                                    out=pt[:, off : off + 128],
                                    in_=pt[:, off : off + 128],
                                    compare_op=AL.is_ge, fill=0.0,
                                    base=0, channel_multiplier=-1,
                                    pattern=[[1, 128]],
                                )
                            pts.append(pt)
                        for g in range(8 * kt // nk, 8 * (kt + 1) // nk):
                            if g < len(ygroups):
                                emit_y_group(*ygroups[g])
                        if prev is not None:
                            emit_avdn(prev)
                        prev = (kt, off, pts)
                    emit_avdn(prev)
                    # (b) denominators, normalize
                    if qc > 0:
                        for h in range(HPC):
                            nc.vector.tensor_add(dn0s[h], dn0s[h], dn1s[h])
                    dnr4 = psSP.tile([128, 512], f32, tag="sp", name="dnr4")
                    for h in range(HPC):
                        nc.tensor.matmul(
                            dnr4[32 * h : 32 * h + 1, :], ones_col_bf, dn0s[h],
                            start=True, stop=True, skip_group_check=True,
                            tile_position=(0, 32 * h),
                        )
                    rds = []
                    with nc.allow_low_precision(reason="tf32 rdenom"):
                        for h in range(HPC):
                            rd = rdp.tile([1, 512], f32r, tag="rd", name="rd")
                            nc.vector.reciprocal(rd, dnr4[32 * h : 32 * h + 1, :])
                            rds.append(rd)
                    rbs = []
                    for h in range(HPC):
                        rb_sb = rbp.tile([128, 512], f32r, tag="rbs", name="rb_sb")
                        nc.gpsimd.partition_broadcast(rb_sb, rds[h])
                        rbs.append(rb_sb)
                    for h in range(HPC):
                        nc.vector.tensor_mul(at_all[:, h, qsl], avs[h], rbs[h])
                for ti in range(4 * (NQ - 1), 4 * NQ):
                    for hc in range(2):
                        emit_y_group(ti, hc)

    nc.compile()
    return nc


def _get_program():
    if "nc" not in _CACHED:
        _CACHED["nc"] = _build_program()
    return _CACHED["nc"]


def _tf32_round(a):
    """Round f32 -> tf32 (10 mantissa bits), nearest-even, on the host."""
    b = np.ascontiguousarray(a, np.float32).view(np.uint32)
    bias = np.uint32(0xFFF) + ((b >> np.uint32(13)) & np.uint32(1))
    return ((b + bias) & np.uint32(0xFFFFE000)).view(np.float32)


def _bf16(a):
    import ml_dtypes

    return np.ascontiguousarray(a, np.float32).astype(ml_dtypes.bfloat16)


def _prep_core_inputs(x, gamma, beta, Wq, Wk, Wv, Wo, core):
    b, g = core // 2, core % 2
    gs = slice(g * F, (g + 1) * F)
    ins = {"xT": _bf16(x[b].T)}
    for name, W in (("q", Wq), ("k", Wk), ("v", Wv)):
        W_eff = W[gs, :] * gamma[None, :]
        w_bf = _bf16(W_eff.T)
        ins["w%sT" % name] = w_bf
        bias = W[gs, :] @ beta
        negws = -w_bf.astype(np.float32).sum(axis=0)
        ins["aux%s" % name] = _tf32_round(np.stack([negws, bias]).astype(np.float32))
    ins["woT"] = _bf16(Wo[:, gs].T)
    return ins


def kernel(x, gamma, beta, Wq, Wk, Wv, Wo, _trace=False):
    from concourse.bass_utils import run_bass_kernel_spmd

    x = np.asarray(x, dtype=np.float32)
    gamma = np.asarray(gamma, dtype=np.float32)
    beta = np.asarray(beta, dtype=np.float32)
    Wq, Wk = np.asarray(Wq, np.float32), np.asarray(Wk, np.float32)
    Wv, Wo = np.asarray(Wv, np.float32), np.asarray(Wo, np.float32)

    nc = _get_program()
    in_maps = [
        _prep_core_inputs(x, gamma, beta, Wq, Wk, Wv, Wo, i) for i in range(NCORES)
    ]
    res = run_bass_kernel_spmd(nc, in_maps, list(range(NCORES)), trace=_trace)
    _CACHED["last_result"] = res
    y = np.empty((B, T, H), np.float32)
    for b in range(B):
        y[b] = (
            res.results[2 * b]["out"].astype(np.float32)
            + res.results[2 * b + 1]["out"].astype(np.float32)
            + x[b]
        )
    return y
